# revision 1
# baseline (speedup 1.0000x reference)
"""DeepSetPred Trainium2 kernel: 3-layer token encoder MLP + segment-sum +
predictor MLP on 8 NeuronCores, with ZERO collectives.

Sharding: the host cuts the (sorted-by-segment) token axis at segment
boundaries, so every segment belongs to exactly one core. Each shard is
padded to a common length with tokens whose one-hot selector row is all
zero (they flow through the encoder but contribute nothing to any segment).
Each core therefore computes the complete segment sums for its own
contiguous range of <=32 segments, runs the predictor on just those rows,
and writes its private slice of the output; the host concatenates.

Layout: tokens on the matmul free dim (features on partitions), fp16
encoder matmuls (2-byte fast weight loads, fp32 PSUM accumulation,
~7e-4 rel err), per-feature bias+tanh fused on the ScalarEngine, the
ragged segment-sum as a one-hot stationary matmul accumulating into a
persistent PSUM bank with the n_s*b3 bias folded in as one K=1 fp32
matmul, and an fp32r predictor.
"""

import numpy as np

import concourse.mybir as mybir
import concourse.tile as tile
from concourse import bacc
from concourse import bass_utils
from concourse.masks import make_identity

# Problem shapes (hardcoded per contract).
T, E, H, C, O = 131072, 256, 512, 256, 32
S = 128            # num segments
N_CORES = 8
TOK = 512          # tokens per inner chunk
G = 1              # chunks per super-chunk (DMA batching granularity)
SCTOK = G * TOK    # 1024
MIN_SLOTS = 32     # baseline segments-per-core capacity
F32 = mybir.dt.float32
F32R = mybir.dt.float32r
F16 = mybir.dt.float16

_CACHE = {}


def _mm(nc, out, lhsT, rhs, start, stop, skip=False):
    nc.tensor.matmul(out, lhsT, rhs,
                     start=start, stop=stop, skip_group_check=skip)


def _build_nc(t_sh, SLOTS):
    assert t_sh % 128 == 0

    nc = bacc.Bacc("TRN2", target_bir_lowering=False, debug=False,
                   num_devices=N_CORES)

    xt_d = nc.dram_tensor("xt", [E, t_sh], F16, kind="ExternalInput")
    sel_d = nc.dram_tensor("sel", [t_sh, SLOTS], F16, kind="ExternalInput")
    cnt_d = nc.dram_tensor("cnt", [1, SLOTS], F32, kind="ExternalInput")
    w1_d = nc.dram_tensor("w1", [E, H], F16, kind="ExternalInput")
    w2_d = nc.dram_tensor("w2", [H, H], F16, kind="ExternalInput")
    w3_d = nc.dram_tensor("w3", [H, C], F16, kind="ExternalInput")
    b1_d = nc.dram_tensor("b1", [H // 128, 128], F32, kind="ExternalInput")
    b2_d = nc.dram_tensor("b2", [H // 128, 128], F32, kind="ExternalInput")
    b3_d = nc.dram_tensor("b3", [1, C], F32, kind="ExternalInput")
    p1_d = nc.dram_tensor("p1", [C, H], F32R, kind="ExternalInput")
    p2_d = nc.dram_tensor("p2", [H, H], F32R, kind="ExternalInput")
    p3_d = nc.dram_tensor("p3", [H, O], F32R, kind="ExternalInput")
    pb1_d = nc.dram_tensor("pb1", [H // 128, 128], F32, kind="ExternalInput")
    pb2_d = nc.dram_tensor("pb2", [H // 128, 128], F32, kind="ExternalInput")
    pb3_d = nc.dram_tensor("pb3", [1, O], F32, kind="ExternalInput")
    out_d = nc.dram_tensor("pred", [SLOTS, O], F32, kind="ExternalOutput")

    EC = E // 128   # 2
    HC = H // 128   # 4
    CC = C // 128   # 2
    TT = TOK // 128  # 4 token sub-tiles per chunk

    with tile.TileContext(nc) as tc:
        with tc.tile_pool(name="wts", bufs=1) as wp, \
             tc.tile_pool(name="xt", bufs=3) as xtp, \
             tc.tile_pool(name="sel", bufs=3) as selp, \
             tc.tile_pool(name="act", bufs=4) as actp, \
             tc.tile_pool(name="small", bufs=1) as smp, \
             tc.tile_pool(name="ps", bufs=2, space="PSUM") as psp, \
             tc.tile_pool(name="psacc", bufs=1, space="PSUM") as psa:

            # warm the ACT tanh table before the scalar queue fills with DMAs
            warm_sb = smp.tile([1, 1], F32, tag="warm", name="warm")
            nc.gpsimd.memset(warm_sb[:], 0.0)
            warm_o = smp.tile([1, 1], F32, tag="warmo", name="warmo")
            nc.scalar.activation(warm_o[:], warm_sb[:],
                                 mybir.ActivationFunctionType.Tanh)

            # ---- resident weights (one batched HWDGE DMA per matrix;
            # encoder weights on the scalar ring, predictor weights on
            # gpsimd so the ACT queue stays clear for tanh) ----
            w1_t = wp.tile([128, EC, HC, 128], F16, tag="w1", name="w1t")
            nc.scalar.dma_start(
                w1_t[:], w1_d.ap().rearrange("(e p) (h q) -> p e h q",
                                             p=128, q=128))
            w1_sb = [[w1_t[:, e, h, :] for h in range(HC)] for e in range(EC)]
            w2_t = wp.tile([128, HC, HC, 128], F16, tag="w2", name="w2t")
            nc.scalar.dma_start(
                w2_t[:], w2_d.ap().rearrange("(k p) (h q) -> p k h q",
                                             p=128, q=128))
            w2_sb = [[w2_t[:, k, h, :] for h in range(HC)] for k in range(HC)]
            w3_t = wp.tile([128, HC, C], F16, tag="w3", name="w3t")
            nc.scalar.dma_start(
                w3_t[:], w3_d.ap().rearrange("(k p) c -> p k c", p=128))
            w3_sb = [w3_t[:, k, :] for k in range(HC)]
            # ---- biases / rows (gpsimd ring; tiny) ----
            b1_sb = smp.tile([128, HC], F32, tag="b1", name="b1")
            nc.gpsimd.dma_start(b1_sb[:], b1_d.ap().rearrange("h p -> p h"))
            b2_sb = smp.tile([128, HC], F32, tag="b2", name="b2")
            nc.gpsimd.dma_start(b2_sb[:], b2_d.ap().rearrange("h p -> p h"))
            pb1_sb = smp.tile([128, HC], F32, tag="pb1", name="pb1")
            nc.gpsimd.dma_start(pb1_sb[:], pb1_d.ap().rearrange("h p -> p h"))
            pb2_sb = smp.tile([128, HC], F32, tag="pb2", name="pb2")
            nc.gpsimd.dma_start(pb2_sb[:], pb2_d.ap().rearrange("h p -> p h"))
            b3row = smp.tile([1, C], F32, tag="b3row", name="b3row")
            nc.gpsimd.dma_start(b3row[:], b3_d.ap())
            pb3row = smp.tile([1, O], F32, tag="pb3row", name="pb3row")
            nc.gpsimd.dma_start(pb3row[:], pb3_d.ap())
            cntrow = smp.tile([1, SLOTS], F32, tag="cntrow", name="cntrow")
            nc.gpsimd.dma_start(cntrow[:], cnt_d.ap())
            ones1 = smp.tile([1, SLOTS], F32, tag="ones1", name="ones1")
            nc.gpsimd.memset(ones1[:], 1.0)
            ident = smp.tile([SLOTS, SLOTS], F32, tag="ident", name="ident")
            make_identity(nc, ident[:])

            p1_t = wp.tile([128, CC, HC, 128], F32R, tag="p1", name="p1t")
            nc.gpsimd.dma_start(
                p1_t[:], p1_d.ap().rearrange("(c p) (h q) -> p c h q",
                                             p=128, q=128))
            p1_sb = [[p1_t[:, c, h, :] for h in range(HC)] for c in range(CC)]
            p2_t = wp.tile([128, HC, HC, 128], F32R, tag="p2", name="p2t")
            nc.gpsimd.dma_start(
                p2_t[:], p2_d.ap().rearrange("(k p) (h q) -> p k h q",
                                             p=128, q=128))
            p2_sb = [[p2_t[:, k, h, :] for h in range(HC)] for k in range(HC)]
            p3_t = wp.tile([128, HC, O], F32R, tag="p3", name="p3t")
            nc.gpsimd.dma_start(
                p3_t[:], p3_d.ap().rearrange("(k p) o -> p k o", p=128))
            p3_sb = [p3_t[:, k, :] for k in range(HC)]

            # ---- persistent segment-sum accumulator enc[slot, c] ----
            enc_ps = psa.tile([SLOTS, C], F32, tag="encacc", name="encacc")
            # enc[slot, c] = counts[slot] * b3[c]  (K=1 fp32 matmul opens it)
            nc.tensor.matmul(enc_ps[:], cntrow[:], b3row[:],
                             start=True, stop=False, skip_group_check=True)

            # ---- main token loop, software-pipelined with a 1-chunk skew:
            # L1(i+1) is emitted before L2/L3/seg(i) so the PE's strict-FIFO
            # queue never head-of-line blocks on the tanh chain ----
            n_full = t_sh // TOK
            tail = t_sh - n_full * TOK
            chunks = [(i * TOK, TOK) for i in range(n_full)]
            if tail:
                chunks.append((n_full * TOK, tail))

            def load_and_l1(base, tok):
                xt_t = xtp.tile([128, EC, tok], F16, tag="xt", name="xt",
                                padded_shape=[128, EC, TOK])
                nc.sync.dma_start(
                    xt_t[:],
                    xt_d.ap()[:, base:base + tok]
                        .rearrange("(e p) t -> p e t", p=128))
                sel_t = selp.tile([128, tok // 128, SLOTS], F16, tag="sel",
                                  name="sel", padded_shape=[128, TT, SLOTS])
                nc.sync.dma_start(
                    sel_t[:],
                    sel_d.ap()[base:base + tok, :]
                         .rearrange("(q p) s -> p q s", p=128))
                h1_t = actp.tile([128, HC, tok], F16, tag="h1", name="h1",
                                 bufs=5, padded_shape=[128, HC, TOK])
                for h in range(HC):
                    ps1 = psp.tile([128, tok], F32, tag="mm", name="mm",
                                   bufs=4, padded_shape=[128, TOK])
                    for e in range(EC):
                        _mm(nc, ps1[:], w1_sb[e][h], xt_t[:, e, :],
                            start=(e == 0), stop=(e == EC - 1), skip=True)
                    nc.scalar.activation(h1_t[:, h, :], ps1[:],
                                         mybir.ActivationFunctionType.Tanh,
                                         bias=b1_sb[:, h:h + 1])
                return sel_t, h1_t

            def l2_l3_seg(sel_t, h1_t, tok, is_last):
                tt = tok // 128
                h2_t = actp.tile([128, HC, tok], F16, tag="h2", name="h2",
                                 padded_shape=[128, HC, TOK])
                for h in range(HC):
                    ps2 = psp.tile([128, tok], F32, tag="mm", name="mm",
                                   bufs=4, padded_shape=[128, TOK])
                    for k in range(HC):
                        _mm(nc, ps2[:], w2_sb[k][h], h1_t[:, k, :],
                            start=(k == 0), stop=(k == HC - 1), skip=True)
                    nc.scalar.activation(h2_t[:, h, :], ps2[:],
                                         mybir.ActivationFunctionType.Tanh,
                                         bias=b2_sb[:, h:h + 1])
                te_sb = actp.tile([128, tt, C], F16, tag="te", name="te",
                                  padded_shape=[128, TT, C])
                for t in range(tt):
                    ps3 = psp.tile([128, C], F32, tag="l3", name="l3", bufs=3)
                    for k in range(HC):
                        _mm(nc, ps3[:], h2_t[:, k, t * 128:(t + 1) * 128],
                            w3_sb[k], start=(k == 0), stop=(k == HC - 1))
                    nc.vector.tensor_copy(te_sb[:, t, :], ps3[:])
                for t in range(tt):
                    last = is_last and (t == tt - 1)
                    _mm(nc, enc_ps[:], sel_t[:, t, :], te_sb[:, t, :],
                        start=False, stop=last, skip=True)

            pend = []
            for ci, (base, tok) in enumerate(chunks):
                pend.append(load_and_l1(base, tok) + (tok,))
                keep = 2 if ci < 2 else 1
                while len(pend) > keep:
                    l2_l3_seg(*pend.pop(0), is_last=False)
            while pend:
                args = pend.pop(0)
                l2_l3_seg(*args, is_last=(len(pend) == 0))

            # ---- predictor on this core's own <=SLOTS segment rows ----
            enc_sb = smp.tile([SLOTS, C], F32, tag="encsb", name="encsb")
            nc.vector.tensor_copy(enc_sb[:], enc_ps[:])
            encT_sb = smp.tile([128, CC, SLOTS], F32R, tag="encT", name="encT")
            for c in range(CC):
                pst = psp.tile([128, SLOTS], F32, tag="l3", name="pst", bufs=3)
                nc.tensor.transpose(pst[:], enc_sb[:, c * 128:(c + 1) * 128],
                                    ident[:])
                nc.vector.tensor_copy(encT_sb[:, c, :], pst[:])

            q1_sb = smp.tile([128, HC, SLOTS], F32R, tag="q1", name="q1")
            for h in range(HC):
                pp1 = psp.tile([128, SLOTS], F32, tag="mm", name="pp1", bufs=4)
                for c in range(CC):
                    _mm(nc, pp1[:], p1_sb[c][h], encT_sb[:, c, :],
                        start=(c == 0), stop=(c == CC - 1))
                nc.scalar.activation(q1_sb[:, h, :], pp1[:],
                                     mybir.ActivationFunctionType.Tanh,
                                     bias=pb1_sb[:, h:h + 1])
            q2_sb = smp.tile([128, HC, SLOTS], F32R, tag="q2", name="q2")
            for h in range(HC):
                pp2 = psp.tile([128, SLOTS], F32, tag="mm", name="pp2", bufs=4)
                for k in range(HC):
                    _mm(nc, pp2[:], p2_sb[k][h], q1_sb[:, k, :],
                        start=(k == 0), stop=(k == HC - 1))
                nc.scalar.activation(q2_sb[:, h, :], pp2[:],
                                     mybir.ActivationFunctionType.Tanh,
                                     bias=pb2_sb[:, h:h + 1])

            # final: pred[slot, o] = q2.T @ P3 + pb3
            ppo = psp.tile([SLOTS, O], F32, tag="l3", name="ppo", bufs=3)
            nc.tensor.matmul(ppo[:], ones1[:], pb3row[:],
                             start=True, stop=False, skip_group_check=True)
            for k in range(HC):
                _mm(nc, ppo[:], q2_sb[:, k, :], p3_sb[k],
                    start=False, stop=(k == HC - 1), skip=True)
            pred_sb = smp.tile([SLOTS, O], F32, tag="pred", name="predsb")
            nc.vector.tensor_copy(pred_sb[:], ppo[:])
            nc.sync.dma_start(out_d.ap(), pred_sb[:])

    nc.compile()
    return nc


def kernel(words, seg_ids, W1, b1, W2, b2, W3, b3,
           P1, pb1, P2, pb2, P3, pb3, batch_size, alpha_iter, **_):
    words = np.asarray(words, dtype=np.float32)
    seg_ids = np.asarray(seg_ids).astype(np.int64)
    assert words.shape == (T, E), words.shape
    bs, ai = int(batch_size), int(alpha_iter)

    # --- host-side index prep: cut the sorted token axis at segment
    # boundaries so each core owns whole segments ---
    counts = np.bincount(seg_ids, minlength=S)[:S]
    starts = np.concatenate([[0], np.cumsum(counts)])   # [S+1]
    cuts = [0]
    for c in range(1, N_CORES):
        tgt = c * T // N_CORES
        j = int(np.searchsorted(starts, tgt, side="left"))
        if j > 0 and tgt - starts[j - 1] < starts[j] - tgt:
            j -= 1
        cuts.append(int(starts[j]))
    cuts.append(T)
    lens = np.diff(cuts)
    t_sh = int(np.ceil(lens.max() / 128) * 128)

    # contiguous segment range owned by each core (covers all of [0, S));
    # empty shards inherit the following shard's start so ranges stay
    # monotone and collectively exhaustive
    seg_lo = [0] * N_CORES
    for c in range(N_CORES - 1, 0, -1):
        if lens[c] > 0:
            seg_lo[c] = int(seg_ids[cuts[c]])
        else:
            seg_lo[c] = S if c == N_CORES - 1 else seg_lo[c + 1]
    seg_hi = seg_lo[1:] + [S]
    slots_needed = max(seg_hi[c] - seg_lo[c] for c in range(N_CORES))
    SLOTS = min(128, max(MIN_SLOTS, ((slots_needed + 31) // 32) * 32))
    assert slots_needed <= SLOTS, (seg_lo, seg_hi)
    assert bs * ai == S

    xt = np.ascontiguousarray(words.T.astype(np.float16))    # [E, T] fp16

    key = ("nc", t_sh, SLOTS)
    if key not in _CACHE:
        _CACHE[key] = _build_nc(t_sh, SLOTS)
    nc = _CACHE[key]

    common = {
        "w1": np.ascontiguousarray(W1, dtype=np.float16),
        "w2": np.ascontiguousarray(W2, dtype=np.float16),
        "w3": np.ascontiguousarray(W3, dtype=np.float16),
        "b1": np.ascontiguousarray(b1, dtype=np.float32).reshape(H // 128, 128),
        "b2": np.ascontiguousarray(b2, dtype=np.float32).reshape(H // 128, 128),
        "b3": np.ascontiguousarray(b3, dtype=np.float32).reshape(1, C),
        "p1": np.ascontiguousarray(P1, dtype=np.float32),
        "p2": np.ascontiguousarray(P2, dtype=np.float32),
        "p3": np.ascontiguousarray(P3, dtype=np.float32),
        "pb1": np.ascontiguousarray(pb1, dtype=np.float32).reshape(H // 128, 128),
        "pb2": np.ascontiguousarray(pb2, dtype=np.float32).reshape(H // 128, 128),
        "pb3": np.ascontiguousarray(pb3, dtype=np.float32).reshape(1, O),
    }
    in_maps = []
    for c in range(N_CORES):
        lo, hi = cuts[c], cuts[c + 1]
        n = hi - lo
        xt_c = np.zeros((E, t_sh), dtype=np.float16)
        xt_c[:, :n] = xt[:, lo:hi]
        sel_c = np.zeros((t_sh, SLOTS), dtype=np.float16)
        sel_c[:n, :] = (seg_ids[lo:hi, None] ==
                        (seg_lo[c] + np.arange(SLOTS))[None, :])
        cnt_c = np.zeros((1, SLOTS), dtype=np.float32)
        nseg = seg_hi[c] - seg_lo[c]
        cnt_c[0, :nseg] = counts[seg_lo[c]:seg_hi[c]]
        in_maps.append({
            **common,
            "xt": xt_c,
            "sel": sel_c,
            "cnt": cnt_c,
        })

    global _LAST_IN_MAPS
    _LAST_IN_MAPS = in_maps
    res = bass_utils.run_bass_kernel_spmd(nc, in_maps,
                                          core_ids=list(range(N_CORES)))
    pred = np.zeros((S, O), dtype=np.float32)
    for c in range(N_CORES):
        nseg = seg_hi[c] - seg_lo[c]
        if nseg > 0:
            pred[seg_lo[c]:seg_hi[c]] = res.results[c]["pred"][:nseg]
    return pred.reshape(bs, ai, O).astype(np.float32)


_LAST_IN_MAPS = None



# revision 12
# speedup vs baseline: 1.1778x; 1.1778x over previous
"""DeepSetPred Trainium2 kernel: 3-layer token encoder MLP + segment-sum +
predictor MLP on 8 NeuronCores, with ZERO collectives.

Sharding: the host cuts the (sorted-by-segment) token axis at segment
boundaries, so every segment belongs to exactly one core. Each shard is
padded to a common length with tokens whose one-hot selector row is all
zero (they flow through the encoder but contribute nothing to any segment).
Each core therefore computes the complete segment sums for its own
contiguous range of <=32 segments, runs the predictor on just those rows,
and writes its private slice of the output; the host concatenates.

Key restructure vs the v1 kernel: the per-token L3 matmul is gone.
Since L3 is linear, segment_sum(h2 @ W3 + b3) = segment_sum(h2) @ W3 +
counts*b3, so we pool h2 directly and apply W3 once per core on the 32
pooled rows. To make the pooled matmul contraction work, L2 is computed
token-major (tokens on PSUM partitions) by using h1 tiles as the
stationary operand and W2 as the moving operand; the per-feature b2 bias
(now along the free axis) is added by the VectorEngine from a
host-broadcast [128, H] bias tile, fused with the PSUM->SBUF eviction
(scalar_tensor_tensor), and tanh runs as one big batched ScalarEngine
instruction per super-chunk. L1 keeps the feature-major layout (bias per
partition) and batches tanh over two 512-token chunks sharing one bias.

PE work per 512 tokens: 8 L1 + 16 L2 + 4 seg matmuls of free dim 512
(14336 cycles) vs 17408 for v1.
"""

import numpy as np

import concourse.mybir as mybir
import concourse.tile as tile
from concourse import bacc
from concourse import bass_utils
from concourse.masks import make_identity

# Problem shapes (hardcoded per contract).
T, E, H, C, O = 131072, 256, 512, 256, 32
S = 128            # num segments
N_CORES = 8
SC = 1024          # tokens per super-chunk (2 PSUM-bank-sized halves)
MIN_SLOTS = 32     # baseline segments-per-core capacity
F32 = mybir.dt.float32
F32R = mybir.dt.float32r
F16 = mybir.dt.float16

_CACHE = {}


def _mm(nc, out, lhsT, rhs, start, stop, skip=False):
    nc.tensor.matmul(out, lhsT, rhs,
                     start=start, stop=stop, skip_group_check=skip)


def _build_nc(t_sh, SLOTS):
    assert t_sh % SC == 0

    nc = bacc.Bacc("TRN2", target_bir_lowering=False, debug=False,
                   num_devices=N_CORES)

    xt_d = nc.dram_tensor("xt", [E, t_sh], F16, kind="ExternalInput")
    sel_d = nc.dram_tensor("sel", [t_sh, SLOTS], F16, kind="ExternalInput")
    cnt_d = nc.dram_tensor("cnt", [1, SLOTS], F32, kind="ExternalInput")
    w1_d = nc.dram_tensor("w1", [E, H], F16, kind="ExternalInput")
    w2_d = nc.dram_tensor("w2", [H, H], F16, kind="ExternalInput")
    w3_d = nc.dram_tensor("w3", [H, C], F16, kind="ExternalInput")
    b1_d = nc.dram_tensor("b1", [H // 128, 128], F32, kind="ExternalInput")
    b2_d = nc.dram_tensor("b2", [128, H], F32, kind="ExternalInput")
    b3_d = nc.dram_tensor("b3", [1, C], F32, kind="ExternalInput")
    p1_d = nc.dram_tensor("p1", [C, H], F32R, kind="ExternalInput")
    p2_d = nc.dram_tensor("p2", [H, H], F32R, kind="ExternalInput")
    p3_d = nc.dram_tensor("p3", [H, O], F32R, kind="ExternalInput")
    pb1_d = nc.dram_tensor("pb1", [H // 128, 128], F32, kind="ExternalInput")
    pb2_d = nc.dram_tensor("pb2", [H // 128, 128], F32, kind="ExternalInput")
    pb3_d = nc.dram_tensor("pb3", [1, O], F32, kind="ExternalInput")
    out_d = nc.dram_tensor("pred", [SLOTS, O], F32, kind="ExternalOutput")

    EC = E // 128   # 2
    HC = H // 128   # 4
    CC = C // 128   # 2
    QC = SC // 128  # 8 token sub-tiles per super-chunk
    n_strips = max(1, 128 // SLOTS) if SLOTS in (32, 64) else 1
    NS = n_strips * SLOTS

    with tile.TileContext(nc) as tc:
        with tc.tile_pool(name="wts", bufs=1) as wp, \
             tc.tile_pool(name="xt", bufs=3) as xtp, \
             tc.tile_pool(name="sel", bufs=4) as selp, \
             tc.tile_pool(name="act", bufs=3) as actp, \
             tc.tile_pool(name="small", bufs=1) as smp, \
             tc.tile_pool(name="ps1", bufs=2, space="PSUM") as psp1, \
             tc.tile_pool(name="ps2", bufs=3, space="PSUM") as psp2, \
             tc.tile_pool(name="psacc", bufs=1, space="PSUM") as psa:

            # warm the ACT tanh table before the scalar queue fills with DMAs
            warm_sb = smp.tile([1, 1], F32, tag="warm", name="warm")
            nc.gpsimd.memset(warm_sb[:], 0.0)
            warm_o = smp.tile([1, 1], F32, tag="warmo", name="warmo")
            nc.scalar.activation(warm_o[:], warm_sb[:],
                                 mybir.ActivationFunctionType.Tanh)

            # ---- resident weights (one batched HWDGE DMA per matrix;
            # encoder weights on the scalar ring, predictor weights on
            # gpsimd so the ACT queue stays clear for tanh) ----
            w1_t = wp.tile([128, EC, HC, 128], F16, tag="w1", name="w1t")
            nc.scalar.dma_start(
                w1_t[:], w1_d.ap().rearrange("(e p) (h q) -> p e h q",
                                             p=128, q=128))
            w1_sb = [[w1_t[:, e, h, :] for h in range(HC)] for e in range(EC)]
            # W2 in moving-operand layout: [p, k, h] with feat_in = k*128+p
            w2_t = wp.tile([128, HC, H], F16, tag="w2", name="w2t")
            nc.scalar.dma_start(
                w2_t[:], w2_d.ap().rearrange("(k p) h -> p k h", p=128))
            w2_mv = [w2_t[:, k, :] for k in range(HC)]
            w3_t = wp.tile([128, HC, C], F16, tag="w3", name="w3t")
            nc.scalar.dma_start(
                w3_t[:], w3_d.ap().rearrange("(k p) c -> p k c", p=128))
            # ---- biases / rows ----
            b1_sb = smp.tile([128, HC], F32, tag="b1", name="b1")
            nc.gpsimd.dma_start(b1_sb[:], b1_d.ap().rearrange("h p -> p h"))
            b2_sb = smp.tile([128, H], F32, tag="b2", name="b2")
            nc.gpsimd.dma_start(b2_sb[:], b2_d.ap())
            pb1_sb = smp.tile([128, HC], F32, tag="pb1", name="pb1")
            nc.gpsimd.dma_start(pb1_sb[:], pb1_d.ap().rearrange("h p -> p h"))
            pb2_sb = smp.tile([128, HC], F32, tag="pb2", name="pb2")
            nc.gpsimd.dma_start(pb2_sb[:], pb2_d.ap().rearrange("h p -> p h"))
            b3row = smp.tile([1, C], F32, tag="b3row", name="b3row")
            nc.gpsimd.dma_start(b3row[:], b3_d.ap())
            pb3row = smp.tile([1, O], F32, tag="pb3row", name="pb3row")
            nc.gpsimd.dma_start(pb3row[:], pb3_d.ap())
            cntrow = smp.tile([1, SLOTS], F32, tag="cntrow", name="cntrow")
            nc.gpsimd.dma_start(cntrow[:], cnt_d.ap())
            ones1 = smp.tile([1, SLOTS], F32, tag="ones1", name="ones1")
            nc.gpsimd.memset(ones1[:], 1.0)
            ident = smp.tile([NS, NS], F32, tag="ident", name="ident")
            make_identity(nc, ident[:])

            p1_t = wp.tile([128, CC, HC, 128], F32R, tag="p1", name="p1t")
            nc.gpsimd.dma_start(
                p1_t[:], p1_d.ap().rearrange("(c p) (h q) -> p c h q",
                                             p=128, q=128))
            p1_sb = [[p1_t[:, c, h, :] for h in range(HC)] for c in range(CC)]
            p2_t = wp.tile([128, HC, HC, 128], F32R, tag="p2", name="p2t")
            nc.gpsimd.dma_start(
                p2_t[:], p2_d.ap().rearrange("(k p) (h q) -> p k h q",
                                             p=128, q=128))
            p2_sb = [[p2_t[:, k, h, :] for h in range(HC)] for k in range(HC)]
            p3_t = wp.tile([128, HC, O], F32R, tag="p3", name="p3t")
            nc.gpsimd.dma_start(
                p3_t[:], p3_d.ap().rearrange("(k p) o -> p k o", p=128))
            p3_sb = [p3_t[:, k, :] for k in range(HC)]

            # ---- persistent segment-sum accumulator, 4 partition strips of
            # SLOTS rows each accumulate disjoint token sub-tiles in one PSUM
            # bank; strip j's matmuls run in column-tiled PE mode (tile
            # position derived from the output base partition) so up to
            # 128//SLOTS one-hot matmuls execute concurrently ----
            enc_ps = psa.tile([NS, H], F32, tag="encacc", name="encacc")
            # dummy opener: zero * ones clears has_written for the bank
            zrow = smp.tile([1, H], F32, tag="zrow", name="zrow")
            nc.gpsimd.memset(zrow[:], 0.0)
            onesN = smp.tile([1, NS], F32, tag="onesN", name="onesN")
            nc.gpsimd.memset(onesN[:], 1.0)
            nc.tensor.matmul(enc_ps[:], onesN[:], zrow[:],
                             start=True, stop=False, skip_group_check=True)

            # super-chunk schedule: full 1024-token units plus an optional
            # trailing 512-token unit (cr = chunks in this unit)
            units = [(si * SC, 2) for si in range(t_sh // SC)]
            if t_sh % SC:
                units.append((t_sh - 512, 1))

            def load_and_l1(base, cr):
                xt_t = xtp.tile([128, EC, cr * 512], F16, tag="xt", name="xt",
                                padded_shape=[128, EC, SC])
                for half in range(cr):
                    nc.sync.dma_start(
                        xt_t[:, :, half * 512:(half + 1) * 512],
                        xt_d.ap()[:, base + half * 512:base + (half + 1) * 512]
                            .rearrange("(e p) t -> p e t", p=128))
                sel_t = selp.tile([128, cr * 4, SLOTS], F16, tag="sel",
                                  name="sel", padded_shape=[128, QC, SLOTS])
                nc.sync.dma_start(
                    sel_t[:],
                    sel_d.ap()[base:base + cr * 512, :]
                         .rearrange("(q p) s -> p q s", p=128))
                h1_t = actp.tile([128, HC, cr, 512], F16, tag="h1", name="h1",
                                 bufs=3, padded_shape=[128, HC, 2, 512])
                for h in range(HC):
                    ps1 = psp1.tile([128, cr, 512], F32, tag="l1", name="l1",
                                    bufs=2, padded_shape=[128, 2, 512])
                    for c in range(cr):
                        for e in range(EC):
                            _mm(nc, ps1[:, c, :], w1_sb[e][h],
                                xt_t[:, e, c * 512:(c + 1) * 512],
                                start=(e == 0), stop=(e == EC - 1), skip=True)
                    nc.scalar.activation(h1_t[:, h, :, :], ps1[:],
                                         mybir.ActivationFunctionType.Tanh,
                                         bias=b1_sb[:, h:h + 1])
                return sel_t, h1_t, cr

            def l2(sel_t, h1_t, cr):
                z2_t = actp.tile([128, cr, 4, 512], F16, tag="z2", name="z2",
                                 bufs=2, padded_shape=[128, 2, 4, 512])
                for c in range(cr):
                    for ts in range(4):
                        ps2 = psp2.tile([128, 512], F32, tag="z2p",
                                        name="z2p", bufs=3)
                        for k in range(HC):
                            _mm(nc, ps2[:],
                                h1_t[:, k, c, ts * 128:(ts + 1) * 128],
                                w2_mv[k],
                                start=(k == 0), stop=(k == HC - 1), skip=True)
                        nc.vector.scalar_tensor_tensor(
                            z2_t[:, c, ts, :], ps2[:], 1.0, b2_sb[:],
                            op0=mybir.AluOpType.mult,
                            op1=mybir.AluOpType.add)
                h2_t = actp.tile([128, cr, 4, 512], F16, tag="h2", name="h2",
                                 bufs=3, padded_shape=[128, 2, 4, 512])
                nc.scalar.activation(h2_t[:], z2_t[:],
                                     mybir.ActivationFunctionType.Tanh)
                return sel_t, h2_t, cr

            def seg(sel_t, h2_t, cr, is_last):
                for c in range(cr):
                    for ts in range(4):
                        q = c * 4 + ts
                        # close each strip's accumulation on its final matmul
                        last = is_last and q >= cr * 4 - n_strips
                        lo = (q % n_strips) * SLOTS
                        nc.tensor.matmul(enc_ps[lo:lo + SLOTS, :],
                                         sel_t[:, q, :], h2_t[:, c, ts, :],
                                         start=False, stop=last,
                                         skip_group_check=True,
                                         tile_position=(0, lo))

            pend1 = []
            pend2 = []
            for base, cr in units:
                pend1.append(load_and_l1(base, cr))
                if len(pend1) > 1:
                    pend2.append(l2(*pend1.pop(0)))
                if len(pend2) > 1:
                    seg(*pend2.pop(0), is_last=False)
            while pend1:
                pend2.append(l2(*pend1.pop(0)))
                if len(pend2) > 1:
                    seg(*pend2.pop(0), is_last=False)
            seg(*pend2.pop(0), is_last=True)

            # ---- epilogue: enc strips -> te[c, slot] -> pred[slot, O] ----
            enc_sb = smp.tile([NS, H], F32, tag="encsb", name="encsb")
            nc.vector.tensor_copy(enc_sb[:], enc_ps[:])
            # transpose to feature-major [128, HC, NS] (f16 for W3 matmul)
            encTf = smp.tile([128, HC, NS], F16, tag="encTf", name="encTf")
            for k in range(HC):
                pst = psp2.tile([128, NS], F32, tag="z2p", name="pst",
                                bufs=3)
                nc.tensor.transpose(pst[:], enc_sb[:, k * 128:(k + 1) * 128],
                                    ident[:])
                nc.vector.tensor_copy(encTf[:, k, :], pst[:])
            # te[c, strip, slot] = sum_k W3[k, c] * encT[k, strip, slot]
            # (+ b3[c]*cnt[slot] folded into strip 0)
            psW3 = psp1.tile([128, CC, NS], F32, tag="l1", name="psW3",
                             bufs=2)
            for ct in range(CC):
                nc.tensor.matmul(psW3[:, ct, 0:SLOTS],
                                 b3row[:, ct * 128:(ct + 1) * 128],
                                 cntrow[:],
                                 start=(ct == 0), stop=False,
                                 skip_group_check=True)
            for ct in range(CC):
                for k in range(HC):
                    _mm(nc, psW3[:, ct, :],
                        w3_t[:, k, ct * 128:(ct + 1) * 128],
                        encTf[:, k, :],
                        start=False, stop=(k == HC - 1), skip=True)
            # sum the strips: te[c, slot] = sum_j te[c, strip j, slot]
            # (DVE has a single PSUM read port, so evacuate to SBUF first)
            encT_sb = smp.tile([128, CC, SLOTS], F32R, tag="encT", name="encT")
            if n_strips == 1:
                for ct in range(CC):
                    nc.vector.tensor_copy(encT_sb[:, ct, :], psW3[:, ct, :])
            else:
                teS = smp.tile([128, CC, NS], F32, tag="teS", name="teS")
                for ct in range(CC):
                    nc.vector.tensor_copy(teS[:, ct, :], psW3[:, ct, :])
                if n_strips == 2:
                    for ct in range(CC):
                        nc.vector.scalar_tensor_tensor(
                            encT_sb[:, ct, :], teS[:, ct, 0:SLOTS], 1.0,
                            teS[:, ct, SLOTS:2 * SLOTS],
                            op0=mybir.AluOpType.mult, op1=mybir.AluOpType.add)
                else:
                    ssum = smp.tile([128, CC, 2, SLOTS], F32, tag="ssum",
                                    name="ssum")
                    for ct in range(CC):
                        for half in range(2):
                            nc.vector.scalar_tensor_tensor(
                                ssum[:, ct, half, :],
                                teS[:, ct, 2 * half * SLOTS:
                                    (2 * half + 1) * SLOTS], 1.0,
                                teS[:, ct, (2 * half + 1) * SLOTS:
                                    (2 * half + 2) * SLOTS],
                                op0=mybir.AluOpType.mult,
                                op1=mybir.AluOpType.add)
                        nc.vector.scalar_tensor_tensor(
                            encT_sb[:, ct, :], ssum[:, ct, 0, :], 1.0,
                            ssum[:, ct, 1, :],
                            op0=mybir.AluOpType.mult, op1=mybir.AluOpType.add)

            # ---- predictor on this core's own <=SLOTS segment rows ----
            q1_sb = smp.tile([128, HC, SLOTS], F32R, tag="q1", name="q1")
            for h in range(HC):
                pp1 = psp2.tile([128, SLOTS], F32, tag="z2p", name="pp1",
                                bufs=3)
                for c in range(CC):
                    _mm(nc, pp1[:], p1_sb[c][h], encT_sb[:, c, :],
                        start=(c == 0), stop=(c == CC - 1))
                nc.scalar.activation(q1_sb[:, h, :], pp1[:],
                                     mybir.ActivationFunctionType.Tanh,
                                     bias=pb1_sb[:, h:h + 1])
            q2_sb = smp.tile([128, HC, SLOTS], F32R, tag="q2", name="q2")
            for h in range(HC):
                pp2 = psp2.tile([128, SLOTS], F32, tag="z2p", name="pp2",
                                bufs=3)
                for k in range(HC):
                    _mm(nc, pp2[:], p2_sb[k][h], q1_sb[:, k, :],
                        start=(k == 0), stop=(k == HC - 1))
                nc.scalar.activation(q2_sb[:, h, :], pp2[:],
                                     mybir.ActivationFunctionType.Tanh,
                                     bias=pb2_sb[:, h:h + 1])

            # final: pred[slot, o] = q2.T @ P3 + pb3
            ppo = psp2.tile([SLOTS, O], F32, tag="z2p", name="ppo", bufs=3)
            nc.tensor.matmul(ppo[:], ones1[:], pb3row[:],
                             start=True, stop=False, skip_group_check=True)
            for k in range(HC):
                _mm(nc, ppo[:], q2_sb[:, k, :], p3_sb[k],
                    start=False, stop=(k == HC - 1), skip=True)
            pred_sb = smp.tile([SLOTS, O], F32, tag="pred", name="predsb")
            nc.vector.tensor_copy(pred_sb[:], ppo[:])
            nc.sync.dma_start(out_d.ap(), pred_sb[:])

    nc.compile()
    return nc


def kernel(words, seg_ids, W1, b1, W2, b2, W3, b3,
           P1, pb1, P2, pb2, P3, pb3, batch_size, alpha_iter, **_):
    words = np.asarray(words, dtype=np.float32)
    seg_ids = np.asarray(seg_ids).astype(np.int64)
    assert words.shape == (T, E), words.shape
    bs, ai = int(batch_size), int(alpha_iter)

    # --- host-side index prep: cut the sorted token axis at segment
    # boundaries so each core owns whole segments ---
    counts = np.bincount(seg_ids, minlength=S)[:S]
    starts = np.concatenate([[0], np.cumsum(counts)])   # [S+1]
    cuts = [0]
    for c in range(1, N_CORES):
        tgt = c * T // N_CORES
        j = int(np.searchsorted(starts, tgt, side="left"))
        if j > 0 and tgt - starts[j - 1] < starts[j] - tgt:
            j -= 1
        cuts.append(int(starts[j]))
    cuts.append(T)
    lens = np.diff(cuts)
    t_sh = int(np.ceil(lens.max() / SC) * SC)

    # contiguous segment range owned by each core (covers all of [0, S));
    # empty shards inherit the following shard's start so ranges stay
    # monotone and collectively exhaustive
    seg_lo = [0] * N_CORES
    for c in range(N_CORES - 1, 0, -1):
        if lens[c] > 0:
            seg_lo[c] = int(seg_ids[cuts[c]])
        else:
            seg_lo[c] = S if c == N_CORES - 1 else seg_lo[c + 1]
    seg_hi = seg_lo[1:] + [S]
    slots_needed = max(seg_hi[c] - seg_lo[c] for c in range(N_CORES))
    SLOTS = min(128, max(MIN_SLOTS, ((slots_needed + 31) // 32) * 32))
    assert slots_needed <= SLOTS, (seg_lo, seg_hi)
    assert bs * ai == S

    xt = np.ascontiguousarray(words.T.astype(np.float16))    # [E, T] fp16

    key = ("nc", t_sh, SLOTS)
    if key not in _CACHE:
        _CACHE[key] = _build_nc(t_sh, SLOTS)
    nc = _CACHE[key]

    common = {
        "w1": np.ascontiguousarray(W1, dtype=np.float16),
        "w2": np.ascontiguousarray(W2, dtype=np.float16),
        "w3": np.ascontiguousarray(W3, dtype=np.float16),
        "b1": np.ascontiguousarray(b1, dtype=np.float32).reshape(H // 128, 128),
        "b2": np.ascontiguousarray(
            np.broadcast_to(np.asarray(b2, dtype=np.float32)[None, :],
                            (128, H))),
        "b3": np.ascontiguousarray(b3, dtype=np.float32).reshape(1, C),
        "p1": np.ascontiguousarray(P1, dtype=np.float32),
        "p2": np.ascontiguousarray(P2, dtype=np.float32),
        "p3": np.ascontiguousarray(P3, dtype=np.float32),
        "pb1": np.ascontiguousarray(pb1, dtype=np.float32).reshape(H // 128, 128),
        "pb2": np.ascontiguousarray(pb2, dtype=np.float32).reshape(H // 128, 128),
        "pb3": np.ascontiguousarray(pb3, dtype=np.float32).reshape(1, O),
    }
    in_maps = []
    for c in range(N_CORES):
        lo, hi = cuts[c], cuts[c + 1]
        n = hi - lo
        xt_c = np.zeros((E, t_sh), dtype=np.float16)
        xt_c[:, :n] = xt[:, lo:hi]
        sel_c = np.zeros((t_sh, SLOTS), dtype=np.float16)
        sel_c[:n, :] = (seg_ids[lo:hi, None] ==
                        (seg_lo[c] + np.arange(SLOTS))[None, :])
        cnt_c = np.zeros((1, SLOTS), dtype=np.float32)
        nseg = seg_hi[c] - seg_lo[c]
        cnt_c[0, :nseg] = counts[seg_lo[c]:seg_hi[c]]
        in_maps.append({
            **common,
            "xt": xt_c,
            "sel": sel_c,
            "cnt": cnt_c,
        })

    global _LAST_IN_MAPS
    _LAST_IN_MAPS = in_maps
    res = bass_utils.run_bass_kernel_spmd(nc, in_maps,
                                          core_ids=list(range(N_CORES)))
    pred = np.zeros((S, O), dtype=np.float32)
    for c in range(N_CORES):
        nseg = seg_hi[c] - seg_lo[c]
        if nseg > 0:
            pred[seg_lo[c]:seg_hi[c]] = res.results[c]["pred"][:nseg]
    return pred.reshape(bs, ai, O).astype(np.float32)


_LAST_IN_MAPS = None


# revision 20
# speedup vs baseline: 1.1811x; 1.0028x over previous
"""DeepSetPred Trainium2 kernel: 3-layer token encoder MLP + segment-sum +
predictor MLP on 8 NeuronCores, with ZERO collectives.

Sharding: the host cuts the (sorted-by-segment) token axis at segment
boundaries, so every segment belongs to exactly one core. Each shard is
padded to a common length with tokens whose one-hot selector row is all
zero (they flow through the encoder but contribute nothing to any segment).
Each core therefore computes the complete segment sums for its own
contiguous range of <=32 segments, runs the predictor on just those rows,
and writes its private slice of the output; the host concatenates.

Key restructure vs the v1 kernel: the per-token L3 matmul is gone.
Since L3 is linear, segment_sum(h2 @ W3 + b3) = segment_sum(h2) @ W3 +
counts*b3, so we pool h2 directly and apply W3 once per core on the 32
pooled rows. To make the pooled matmul contraction work, L2 is computed
token-major (tokens on PSUM partitions) by using h1 tiles as the
stationary operand and W2 as the moving operand; the per-feature b2 bias
(now along the free axis) is added by the VectorEngine from a
host-broadcast [128, H] bias tile, fused with the PSUM->SBUF eviction
(scalar_tensor_tensor), and tanh runs as one big batched ScalarEngine
instruction per super-chunk. L1 keeps the feature-major layout (bias per
partition) and batches tanh over two 512-token chunks sharing one bias.

PE work per 512 tokens: 8 L1 + 16 L2 + 4 seg matmuls of free dim 512
(14336 cycles) vs 17408 for v1.
"""

import numpy as np

import concourse.mybir as mybir
import concourse.tile as tile
from concourse import bacc
from concourse import bass_utils
from concourse.masks import make_identity

# Problem shapes (hardcoded per contract).
T, E, H, C, O = 131072, 256, 512, 256, 32
S = 128            # num segments
N_CORES = 8
SC = 1024          # tokens per super-chunk (2 PSUM-bank-sized halves)
MIN_SLOTS = 32     # baseline segments-per-core capacity
F32 = mybir.dt.float32
F32R = mybir.dt.float32r
F16 = mybir.dt.float16

_CACHE = {}


def _mm(nc, out, lhsT, rhs, start, stop, skip=False):
    nc.tensor.matmul(out, lhsT, rhs,
                     start=start, stop=stop, skip_group_check=skip)


def _build_nc(t_sh, SLOTS):
    assert t_sh % 512 == 0

    nc = bacc.Bacc("TRN2", target_bir_lowering=False, debug=False,
                   num_devices=N_CORES)

    xt_d = nc.dram_tensor("xt", [E, t_sh], F16, kind="ExternalInput")
    sel_d = nc.dram_tensor("sel", [t_sh, SLOTS], F16, kind="ExternalInput")
    cnt_d = nc.dram_tensor("cnt", [1, SLOTS], F32, kind="ExternalInput")
    w1_d = nc.dram_tensor("w1", [E, H], F16, kind="ExternalInput")
    w2_d = nc.dram_tensor("w2", [H, H], F16, kind="ExternalInput")
    w3_d = nc.dram_tensor("w3", [H, C], F16, kind="ExternalInput")
    b1_d = nc.dram_tensor("b1", [H // 128, 128], F32, kind="ExternalInput")
    b2_d = nc.dram_tensor("b2", [128, H], F32, kind="ExternalInput")
    b3_d = nc.dram_tensor("b3", [1, C], F32, kind="ExternalInput")
    p1_d = nc.dram_tensor("p1", [C, H], F32R, kind="ExternalInput")
    p2_d = nc.dram_tensor("p2", [H, H], F32R, kind="ExternalInput")
    p3_d = nc.dram_tensor("p3", [H, O], F32R, kind="ExternalInput")
    pb1_d = nc.dram_tensor("pb1", [H // 128, 128], F32, kind="ExternalInput")
    pb2_d = nc.dram_tensor("pb2", [H // 128, 128], F32, kind="ExternalInput")
    pb3_d = nc.dram_tensor("pb3", [1, O], F32, kind="ExternalInput")
    out_d = nc.dram_tensor("pred", [SLOTS, O], F32, kind="ExternalOutput")

    EC = E // 128   # 2
    HC = H // 128   # 4
    CC = C // 128   # 2
    QC = SC // 128  # 8 token sub-tiles per super-chunk
    n_strips = max(1, 128 // SLOTS) if SLOTS in (32, 64) else 1
    NS = n_strips * SLOTS

    with tile.TileContext(nc) as tc:
        with tc.tile_pool(name="wts", bufs=1) as wp, \
             tc.tile_pool(name="xt", bufs=3) as xtp, \
             tc.tile_pool(name="sel", bufs=4) as selp, \
             tc.tile_pool(name="act", bufs=3) as actp, \
             tc.tile_pool(name="small", bufs=1) as smp, \
             tc.tile_pool(name="ps1", bufs=2, space="PSUM") as psp1, \
             tc.tile_pool(name="ps2", bufs=3, space="PSUM") as psp2, \
             tc.tile_pool(name="psacc", bufs=1, space="PSUM") as psa:

            # warm the ACT tanh table before the scalar queue fills with DMAs
            warm_sb = smp.tile([1, 1], F32, tag="warm", name="warm")
            nc.gpsimd.memset(warm_sb[:], 0.0)
            warm_o = smp.tile([1, 1], F32, tag="warmo", name="warmo")
            nc.scalar.activation(warm_o[:], warm_sb[:],
                                 mybir.ActivationFunctionType.Tanh)

            # ---- resident weights (one batched HWDGE DMA per matrix;
            # encoder weights on the scalar ring, predictor weights on
            # gpsimd so the ACT queue stays clear for tanh) ----
            w1_t = wp.tile([128, EC, HC, 128], F16, tag="w1", name="w1t")
            nc.scalar.dma_start(
                w1_t[:], w1_d.ap().rearrange("(e p) (h q) -> p e h q",
                                             p=128, q=128))
            w1_sb = [[w1_t[:, e, h, :] for h in range(HC)] for e in range(EC)]
            # W2 in moving-operand layout: [p, k, h] with feat_in = k*128+p
            w2_t = wp.tile([128, HC, H], F16, tag="w2", name="w2t")
            nc.scalar.dma_start(
                w2_t[:], w2_d.ap().rearrange("(k p) h -> p k h", p=128))
            w2_mv = [w2_t[:, k, :] for k in range(HC)]
            w3_t = wp.tile([128, HC, C], F16, tag="w3", name="w3t")
            nc.scalar.dma_start(
                w3_t[:], w3_d.ap().rearrange("(k p) c -> p k c", p=128))
            # ---- biases / rows ----
            b1_sb = smp.tile([128, HC], F32, tag="b1", name="b1")
            nc.gpsimd.dma_start(b1_sb[:], b1_d.ap().rearrange("h p -> p h"))
            b2_sb = smp.tile([128, H], F32, tag="b2", name="b2")
            nc.gpsimd.dma_start(b2_sb[:], b2_d.ap())
            pb1_sb = smp.tile([128, HC], F32, tag="pb1", name="pb1")
            nc.gpsimd.dma_start(pb1_sb[:], pb1_d.ap().rearrange("h p -> p h"))
            pb2_sb = smp.tile([128, HC], F32, tag="pb2", name="pb2")
            nc.gpsimd.dma_start(pb2_sb[:], pb2_d.ap().rearrange("h p -> p h"))
            b3row = smp.tile([1, C], F32, tag="b3row", name="b3row")
            nc.gpsimd.dma_start(b3row[:], b3_d.ap())
            pb3row = smp.tile([1, O], F32, tag="pb3row", name="pb3row")
            nc.gpsimd.dma_start(pb3row[:], pb3_d.ap())
            cntrow = smp.tile([1, SLOTS], F32, tag="cntrow", name="cntrow")
            nc.gpsimd.dma_start(cntrow[:], cnt_d.ap())
            ones1 = smp.tile([1, SLOTS], F32, tag="ones1", name="ones1")
            nc.gpsimd.memset(ones1[:], 1.0)
            ident = smp.tile([NS, NS], F32, tag="ident", name="ident")
            make_identity(nc, ident[:])

            # ---- persistent segment-sum accumulator, 4 partition strips of
            # SLOTS rows each accumulate disjoint token sub-tiles in one PSUM
            # bank; strip j's matmuls run in column-tiled PE mode (tile
            # position derived from the output base partition) so up to
            # 128//SLOTS one-hot matmuls execute concurrently ----
            enc_ps = psa.tile([NS, H], F32, tag="encacc", name="encacc")
            # dummy opener: zero * ones clears has_written for the bank
            zrow = smp.tile([1, H], F32, tag="zrow", name="zrow")
            nc.gpsimd.memset(zrow[:], 0.0)
            onesN = smp.tile([1, NS], F32, tag="onesN", name="onesN")
            nc.gpsimd.memset(onesN[:], 1.0)
            nc.tensor.matmul(enc_ps[:], onesN[:], zrow[:],
                             start=True, stop=False, skip_group_check=True)

            # super-chunk schedule: full 1024-token units plus an optional
            # trailing 512-token unit (cr = chunks in this unit)
            units = [(si * SC, 2) for si in range(t_sh // SC)]
            if t_sh % SC:
                units.append((t_sh - 512, 1))

            def load_and_l1(base, cr):
                xt_t = xtp.tile([128, EC, cr * 512], F16, tag="xt", name="xt",
                                padded_shape=[128, EC, SC])
                # halves on different DMA rings so the first L1 matmul's
                # input lands in half the time
                nc.sync.dma_start(
                    xt_t[:, :, 0:512],
                    xt_d.ap()[:, base:base + 512]
                        .rearrange("(e p) t -> p e t", p=128))
                if cr > 1:
                    nc.gpsimd.dma_start(
                        xt_t[:, :, 512:1024],
                        xt_d.ap()[:, base + 512:base + 1024]
                            .rearrange("(e p) t -> p e t", p=128))
                sel_t = selp.tile([128, cr * 4, SLOTS], F16, tag="sel",
                                  name="sel", padded_shape=[128, QC, SLOTS])
                nc.sync.dma_start(
                    sel_t[:],
                    sel_d.ap()[base:base + cr * 512, :]
                         .rearrange("(q p) s -> p q s", p=128))
                h1_t = actp.tile([128, HC, cr, 512], F16, tag="h1", name="h1",
                                 bufs=3, padded_shape=[128, HC, 2, 512])
                for h in range(HC):
                    for c in range(cr):
                        ps1 = psp1.tile([128, 512], F32, tag="l1", name="l1",
                                        bufs=5)
                        for e in range(EC):
                            _mm(nc, ps1[:], w1_sb[e][h],
                                xt_t[:, e, c * 512:(c + 1) * 512],
                                start=(e == 0), stop=(e == EC - 1), skip=True)
                        nc.scalar.activation(h1_t[:, h, c, :], ps1[:],
                                             mybir.ActivationFunctionType.Tanh,
                                             bias=b1_sb[:, h:h + 1])
                return sel_t, h1_t, cr

            def l2(sel_t, h1_t, cr):
                z2_t = actp.tile([128, cr, 4, 512], F16, tag="z2", name="z2",
                                 bufs=2, padded_shape=[128, 2, 4, 512])
                for c in range(cr):
                    for ts in range(4):
                        ps2 = psp2.tile([128, 512], F32, tag="z2p",
                                        name="z2p", bufs=2)
                        for k in range(HC):
                            _mm(nc, ps2[:],
                                h1_t[:, k, c, ts * 128:(ts + 1) * 128],
                                w2_mv[k],
                                start=(k == 0), stop=(k == HC - 1), skip=True)
                        nc.vector.scalar_tensor_tensor(
                            z2_t[:, c, ts, :], ps2[:], 1.0, b2_sb[:],
                            op0=mybir.AluOpType.mult,
                            op1=mybir.AluOpType.add)
                h2_t = actp.tile([128, cr, 4, 512], F16, tag="h2", name="h2",
                                 bufs=3, padded_shape=[128, 2, 4, 512])
                nc.scalar.activation(h2_t[:], z2_t[:],
                                     mybir.ActivationFunctionType.Tanh)
                return sel_t, h2_t, cr

            def seg(sel_t, h2_t, cr, is_last):
                for c in range(cr):
                    for ts in range(4):
                        q = c * 4 + ts
                        # close each strip's accumulation on its final matmul
                        last = is_last and q >= cr * 4 - n_strips
                        lo = (q % n_strips) * SLOTS
                        nc.tensor.matmul(enc_ps[lo:lo + SLOTS, :],
                                         sel_t[:, q, :], h2_t[:, c, ts, :],
                                         start=False, stop=last,
                                         skip_group_check=True,
                                         tile_position=(0, lo))

            pend1 = []
            pend2 = []
            for base, cr in units:
                pend1.append(load_and_l1(base, cr))
                if len(pend1) > 1:
                    pend2.append(l2(*pend1.pop(0)))
                if len(pend2) > 1:
                    seg(*pend2.pop(0), is_last=False)
            while pend1:
                pend2.append(l2(*pend1.pop(0)))
                if len(pend2) > 1:
                    seg(*pend2.pop(0), is_last=False)
            seg(*pend2.pop(0), is_last=True)

            p1_t = wp.tile([128, CC, HC, 128], F32R, tag="p1", name="p1t")
            nc.gpsimd.dma_start(
                p1_t[:], p1_d.ap().rearrange("(c p) (h q) -> p c h q",
                                             p=128, q=128))
            p1_sb = [[p1_t[:, c, h, :] for h in range(HC)] for c in range(CC)]
            p2_t = wp.tile([128, HC, HC, 128], F32R, tag="p2", name="p2t")
            nc.gpsimd.dma_start(
                p2_t[:], p2_d.ap().rearrange("(k p) (h q) -> p k h q",
                                             p=128, q=128))
            p2_sb = [[p2_t[:, k, h, :] for h in range(HC)] for k in range(HC)]
            p3_t = wp.tile([128, HC, O], F32R, tag="p3", name="p3t")
            nc.gpsimd.dma_start(
                p3_t[:], p3_d.ap().rearrange("(k p) o -> p k o", p=128))
            p3_sb = [p3_t[:, k, :] for k in range(HC)]

            # ---- epilogue: enc strips -> te[c, slot] -> pred[slot, O] ----
            enc_sb = smp.tile([NS, H], F32, tag="encsb", name="encsb")
            nc.vector.tensor_copy(enc_sb[:], enc_ps[:])
            # transpose to feature-major [128, HC, NS] (f16 for W3 matmul)
            encTf = smp.tile([128, HC, NS], F16, tag="encTf", name="encTf")
            for k in range(HC):
                pst = psp2.tile([128, NS], F32, tag="z2p", name="pst",
                                bufs=2)
                nc.tensor.transpose(pst[:], enc_sb[:, k * 128:(k + 1) * 128],
                                    ident[:])
                nc.vector.tensor_copy(encTf[:, k, :], pst[:])
            # te[c, strip, slot] = sum_k W3[k, c] * encT[k, strip, slot]
            # (+ b3[c]*cnt[slot] folded into strip 0)
            psW3 = psp1.tile([128, CC, NS], F32, tag="l1", name="psW3",
                             bufs=5)
            for ct in range(CC):
                nc.tensor.matmul(psW3[:, ct, 0:SLOTS],
                                 b3row[:, ct * 128:(ct + 1) * 128],
                                 cntrow[:],
                                 start=(ct == 0), stop=False,
                                 skip_group_check=True)
            for ct in range(CC):
                for k in range(HC):
                    _mm(nc, psW3[:, ct, :],
                        w3_t[:, k, ct * 128:(ct + 1) * 128],
                        encTf[:, k, :],
                        start=False, stop=(k == HC - 1), skip=True)
            # sum the strips: te[c, slot] = sum_j te[c, strip j, slot]
            # (DVE has a single PSUM read port, so evacuate to SBUF first)
            encT_sb = smp.tile([128, CC, SLOTS], F32R, tag="encT", name="encT")
            if n_strips == 1:
                for ct in range(CC):
                    nc.vector.tensor_copy(encT_sb[:, ct, :], psW3[:, ct, :])
            else:
                teS = smp.tile([128, CC, NS], F32, tag="teS", name="teS")
                for ct in range(CC):
                    nc.vector.tensor_copy(teS[:, ct, :], psW3[:, ct, :])
                if n_strips == 2:
                    for ct in range(CC):
                        nc.vector.scalar_tensor_tensor(
                            encT_sb[:, ct, :], teS[:, ct, 0:SLOTS], 1.0,
                            teS[:, ct, SLOTS:2 * SLOTS],
                            op0=mybir.AluOpType.mult, op1=mybir.AluOpType.add)
                else:
                    ssum = smp.tile([128, CC, 2, SLOTS], F32, tag="ssum",
                                    name="ssum")
                    for ct in range(CC):
                        for half in range(2):
                            nc.vector.scalar_tensor_tensor(
                                ssum[:, ct, half, :],
                                teS[:, ct, 2 * half * SLOTS:
                                    (2 * half + 1) * SLOTS], 1.0,
                                teS[:, ct, (2 * half + 1) * SLOTS:
                                    (2 * half + 2) * SLOTS],
                                op0=mybir.AluOpType.mult,
                                op1=mybir.AluOpType.add)
                        nc.vector.scalar_tensor_tensor(
                            encT_sb[:, ct, :], ssum[:, ct, 0, :], 1.0,
                            ssum[:, ct, 1, :],
                            op0=mybir.AluOpType.mult, op1=mybir.AluOpType.add)

            # ---- predictor on this core's own <=SLOTS segment rows ----
            q1_sb = smp.tile([128, HC, SLOTS], F32R, tag="q1", name="q1")
            for h in range(HC):
                pp1 = psp2.tile([128, SLOTS], F32, tag="z2p", name="pp1",
                                bufs=2)
                for c in range(CC):
                    _mm(nc, pp1[:], p1_sb[c][h], encT_sb[:, c, :],
                        start=(c == 0), stop=(c == CC - 1))
                nc.scalar.activation(q1_sb[:, h, :], pp1[:],
                                     mybir.ActivationFunctionType.Tanh,
                                     bias=pb1_sb[:, h:h + 1])
            q2_sb = smp.tile([128, HC, SLOTS], F32R, tag="q2", name="q2")
            for h in range(HC):
                pp2 = psp2.tile([128, SLOTS], F32, tag="z2p", name="pp2",
                                bufs=2)
                for k in range(HC):
                    _mm(nc, pp2[:], p2_sb[k][h], q1_sb[:, k, :],
                        start=(k == 0), stop=(k == HC - 1))
                nc.scalar.activation(q2_sb[:, h, :], pp2[:],
                                     mybir.ActivationFunctionType.Tanh,
                                     bias=pb2_sb[:, h:h + 1])

            # final: pred[slot, o] = q2.T @ P3 + pb3
            ppo = psp2.tile([SLOTS, O], F32, tag="z2p", name="ppo", bufs=2)
            nc.tensor.matmul(ppo[:], ones1[:], pb3row[:],
                             start=True, stop=False, skip_group_check=True)
            for k in range(HC):
                _mm(nc, ppo[:], q2_sb[:, k, :], p3_sb[k],
                    start=False, stop=(k == HC - 1), skip=True)
            pred_sb = smp.tile([SLOTS, O], F32, tag="pred", name="predsb")
            nc.vector.tensor_copy(pred_sb[:], ppo[:])
            nc.sync.dma_start(out_d.ap(), pred_sb[:])

    nc.compile()
    return nc


def kernel(words, seg_ids, W1, b1, W2, b2, W3, b3,
           P1, pb1, P2, pb2, P3, pb3, batch_size, alpha_iter, **_):
    words = np.asarray(words, dtype=np.float32)
    seg_ids = np.asarray(seg_ids).astype(np.int64)
    assert words.shape == (T, E), words.shape
    bs, ai = int(batch_size), int(alpha_iter)

    # --- host-side index prep: cut the sorted token axis at segment
    # boundaries so each core owns whole segments ---
    counts = np.bincount(seg_ids, minlength=S)[:S]
    starts = np.concatenate([[0], np.cumsum(counts)])   # [S+1]
    cuts = [0]
    for c in range(1, N_CORES):
        tgt = c * T // N_CORES
        j = int(np.searchsorted(starts, tgt, side="left"))
        if j > 0 and tgt - starts[j - 1] < starts[j] - tgt:
            j -= 1
        cuts.append(int(starts[j]))
    cuts.append(T)
    lens = np.diff(cuts)
    t_sh = int(np.ceil(lens.max() / 512) * 512)

    # contiguous segment range owned by each core (covers all of [0, S));
    # empty shards inherit the following shard's start so ranges stay
    # monotone and collectively exhaustive
    seg_lo = [0] * N_CORES
    for c in range(N_CORES - 1, 0, -1):
        if lens[c] > 0:
            seg_lo[c] = int(seg_ids[cuts[c]])
        else:
            seg_lo[c] = S if c == N_CORES - 1 else seg_lo[c + 1]
    seg_hi = seg_lo[1:] + [S]
    slots_needed = max(seg_hi[c] - seg_lo[c] for c in range(N_CORES))
    SLOTS = min(128, max(MIN_SLOTS, ((slots_needed + 31) // 32) * 32))
    assert slots_needed <= SLOTS, (seg_lo, seg_hi)
    assert bs * ai == S

    xt = np.ascontiguousarray(words.T.astype(np.float16))    # [E, T] fp16

    key = ("nc", t_sh, SLOTS)
    if key not in _CACHE:
        _CACHE[key] = _build_nc(t_sh, SLOTS)
    nc = _CACHE[key]

    common = {
        "w1": np.ascontiguousarray(W1, dtype=np.float16),
        "w2": np.ascontiguousarray(W2, dtype=np.float16),
        "w3": np.ascontiguousarray(W3, dtype=np.float16),
        "b1": np.ascontiguousarray(b1, dtype=np.float32).reshape(H // 128, 128),
        "b2": np.ascontiguousarray(
            np.broadcast_to(np.asarray(b2, dtype=np.float32)[None, :],
                            (128, H))),
        "b3": np.ascontiguousarray(b3, dtype=np.float32).reshape(1, C),
        "p1": np.ascontiguousarray(P1, dtype=np.float32),
        "p2": np.ascontiguousarray(P2, dtype=np.float32),
        "p3": np.ascontiguousarray(P3, dtype=np.float32),
        "pb1": np.ascontiguousarray(pb1, dtype=np.float32).reshape(H // 128, 128),
        "pb2": np.ascontiguousarray(pb2, dtype=np.float32).reshape(H // 128, 128),
        "pb3": np.ascontiguousarray(pb3, dtype=np.float32).reshape(1, O),
    }
    in_maps = []
    for c in range(N_CORES):
        lo, hi = cuts[c], cuts[c + 1]
        n = hi - lo
        xt_c = np.zeros((E, t_sh), dtype=np.float16)
        xt_c[:, :n] = xt[:, lo:hi]
        sel_c = np.zeros((t_sh, SLOTS), dtype=np.float16)
        sel_c[:n, :] = (seg_ids[lo:hi, None] ==
                        (seg_lo[c] + np.arange(SLOTS))[None, :])
        cnt_c = np.zeros((1, SLOTS), dtype=np.float32)
        nseg = seg_hi[c] - seg_lo[c]
        cnt_c[0, :nseg] = counts[seg_lo[c]:seg_hi[c]]
        in_maps.append({
            **common,
            "xt": xt_c,
            "sel": sel_c,
            "cnt": cnt_c,
        })

    global _LAST_IN_MAPS
    _LAST_IN_MAPS = in_maps
    res = bass_utils.run_bass_kernel_spmd(nc, in_maps,
                                          core_ids=list(range(N_CORES)))
    pred = np.zeros((S, O), dtype=np.float32)
    for c in range(N_CORES):
        nseg = seg_hi[c] - seg_lo[c]
        if nseg > 0:
            pred[seg_lo[c]:seg_hi[c]] = res.results[c]["pred"][:nseg]
    return pred.reshape(bs, ai, O).astype(np.float32)


_LAST_IN_MAPS = None


# revision 23
# speedup vs baseline: 1.2201x; 1.0330x over previous
"""DeepSetPred Trainium2 kernel: 3-layer token encoder MLP + segment-sum +
predictor MLP on 8 NeuronCores, with ZERO collectives.

Sharding: the host cuts the (sorted-by-segment) token axis at segment
boundaries, so every segment belongs to exactly one core. Each shard is
padded to a common length with tokens whose one-hot selector row is all
zero (they flow through the encoder but contribute nothing to any segment).
Each core therefore computes the complete segment sums for its own
contiguous range of <=32 segments, runs the predictor on just those rows,
and writes its private slice of the output; the host concatenates.

Key restructure vs the v1 kernel: the per-token L3 matmul is gone.
Since L3 is linear, segment_sum(h2 @ W3 + b3) = segment_sum(h2) @ W3 +
counts*b3, so we pool h2 directly and apply W3 once per core on the 32
pooled rows. To make the pooled matmul contraction work, L2 is computed
token-major (tokens on PSUM partitions) by using h1 tiles as the
stationary operand and W2 as the moving operand; the per-feature b2 bias
(now along the free axis) is added by the VectorEngine from a
host-broadcast [128, H] bias tile, fused with the PSUM->SBUF eviction
(scalar_tensor_tensor), and tanh runs as one big batched ScalarEngine
instruction per super-chunk. L1 keeps the feature-major layout (bias per
partition) and batches tanh over two 512-token chunks sharing one bias.

PE work per 512 tokens: 8 L1 + 16 L2 + 4 seg matmuls of free dim 512
(14336 cycles) vs 17408 for v1.
"""

import numpy as np

import concourse.mybir as mybir
import concourse.tile as tile
from concourse import bacc
from concourse import bass_utils
from concourse.masks import make_identity

# Problem shapes (hardcoded per contract).
T, E, H, C, O = 131072, 256, 512, 256, 32
S = 128            # num segments
N_CORES = 8
SC = 1024          # tokens per super-chunk (2 PSUM-bank-sized halves)
MIN_SLOTS = 32     # baseline segments-per-core capacity
F32 = mybir.dt.float32
F32R = mybir.dt.float32r
F16 = mybir.dt.float16

_CACHE = {}


def _mm(nc, out, lhsT, rhs, start, stop, skip=False):
    nc.tensor.matmul(out, lhsT, rhs,
                     start=start, stop=stop, skip_group_check=skip)


def _build_nc(t_sh, SLOTS):
    assert t_sh % 512 == 0

    nc = bacc.Bacc("TRN2", target_bir_lowering=False, debug=False,
                   num_devices=N_CORES)

    xt_d = nc.dram_tensor("xt", [E, t_sh], F16, kind="ExternalInput")
    sel_d = nc.dram_tensor("sel", [t_sh, 128], F16, kind="ExternalInput")
    cnt_d = nc.dram_tensor("cnt", [1, SLOTS], F32, kind="ExternalInput")
    w1_d = nc.dram_tensor("w1", [E, H], F16, kind="ExternalInput")
    w2_d = nc.dram_tensor("w2", [H, H], F16, kind="ExternalInput")
    w3_d = nc.dram_tensor("w3", [H, C], F16, kind="ExternalInput")
    b1_d = nc.dram_tensor("b1", [H // 128, 128], F32, kind="ExternalInput")
    b2_d = nc.dram_tensor("b2", [128, H], F32, kind="ExternalInput")
    b3_d = nc.dram_tensor("b3", [1, C], F32, kind="ExternalInput")
    p1_d = nc.dram_tensor("p1", [C, H], F32R, kind="ExternalInput")
    p2_d = nc.dram_tensor("p2", [H, H], F32R, kind="ExternalInput")
    p3_d = nc.dram_tensor("p3", [H, O], F32R, kind="ExternalInput")
    pb1_d = nc.dram_tensor("pb1", [H // 128, 128], F32, kind="ExternalInput")
    pb2_d = nc.dram_tensor("pb2", [H // 128, 128], F32, kind="ExternalInput")
    pb3_d = nc.dram_tensor("pb3", [1, O], F32, kind="ExternalInput")
    out_d = nc.dram_tensor("pred", [SLOTS, O], F32, kind="ExternalOutput")

    EC = E // 128   # 2
    HC = H // 128   # 4
    CC = C // 128   # 2
    QC = SC // 128  # 8 token sub-tiles per super-chunk
    NS = 128       # seg-sum accumulator height (sel zero-padded to 128)

    with tile.TileContext(nc) as tc:
        with tc.tile_pool(name="wts", bufs=1) as wp, \
             tc.tile_pool(name="xt", bufs=3) as xtp, \
             tc.tile_pool(name="sel", bufs=4) as selp, \
             tc.tile_pool(name="act", bufs=3) as actp, \
             tc.tile_pool(name="small", bufs=1) as smp, \
             tc.tile_pool(name="ps1", bufs=2, space="PSUM") as psp1, \
             tc.tile_pool(name="ps2", bufs=3, space="PSUM") as psp2, \
             tc.tile_pool(name="psacc", bufs=1, space="PSUM") as psa:

            # warm the ACT tanh table before the scalar queue fills with DMAs
            warm_sb = smp.tile([1, 1], F32, tag="warm", name="warm")
            nc.gpsimd.memset(warm_sb[:], 0.0)
            warm_o = smp.tile([1, 1], F32, tag="warmo", name="warmo")
            nc.scalar.activation(warm_o[:], warm_sb[:],
                                 mybir.ActivationFunctionType.Tanh)

            # ---- resident weights (one batched HWDGE DMA per matrix;
            # encoder weights on the scalar ring, predictor weights on
            # gpsimd so the ACT queue stays clear for tanh) ----
            w1_t = wp.tile([128, EC, HC, 128], F16, tag="w1", name="w1t")
            nc.sync.dma_start(
                w1_t[:], w1_d.ap().rearrange("(e p) (h q) -> p e h q",
                                             p=128, q=128))
            w1_sb = [[w1_t[:, e, h, :] for h in range(HC)] for e in range(EC)]
            # W2 in moving-operand layout: [p, k, h] with feat_in = k*128+p
            w2_t = wp.tile([128, HC, H], F16, tag="w2", name="w2t")
            nc.scalar.dma_start(
                w2_t[:], w2_d.ap().rearrange("(k p) h -> p k h", p=128))
            w2_mv = [w2_t[:, k, :] for k in range(HC)]
            w3_t = wp.tile([128, HC, C], F16, tag="w3", name="w3t")
            nc.scalar.dma_start(
                w3_t[:], w3_d.ap().rearrange("(k p) c -> p k c", p=128))
            # ---- biases / rows ----
            b1_sb = smp.tile([128, HC], F32, tag="b1", name="b1")
            nc.gpsimd.dma_start(b1_sb[:], b1_d.ap().rearrange("h p -> p h"))
            b2_sb = smp.tile([128, H], F32, tag="b2", name="b2")
            pb1_sb = smp.tile([128, HC], F32, tag="pb1", name="pb1")
            pb2_sb = smp.tile([128, HC], F32, tag="pb2", name="pb2")
            b3row = smp.tile([1, C], F32, tag="b3row", name="b3row")
            pb3row = smp.tile([1, O], F32, tag="pb3row", name="pb3row")
            cntrow = smp.tile([1, SLOTS], F32, tag="cntrow", name="cntrow")
            ones1 = smp.tile([1, SLOTS], F32, tag="ones1", name="ones1")
            nc.gpsimd.memset(ones1[:], 1.0)
            ident = smp.tile([NS, NS], F32, tag="ident", name="ident")
            make_identity(nc, ident[:])

            # ---- persistent segment-sum accumulator in one PSUM bank.
            # sel is zero-padded to 128 one-hot columns so the seg matmuls
            # keep the PE in full 128x128 mode (no tile-mode switches) ----
            enc_ps = psa.tile([NS, H], F32, tag="encacc", name="encacc")
            # dummy opener: zero * ones clears has_written for the bank
            zrow = smp.tile([1, H], F32, tag="zrow", name="zrow")
            nc.gpsimd.memset(zrow[:], 0.0)
            onesN = smp.tile([1, NS], F32, tag="onesN", name="onesN")
            nc.gpsimd.memset(onesN[:], 1.0)
            nc.tensor.matmul(enc_ps[:], onesN[:], zrow[:],
                             start=True, stop=False, skip_group_check=True)

            # super-chunk schedule: full 1024-token units plus an optional
            # trailing 512-token unit (cr = chunks in this unit)
            units = [(si * SC, 2) for si in range(t_sh // SC)]
            if t_sh % SC:
                units.append((t_sh - 512, 1))

            def load_and_l1(base, cr):
                xt_t = xtp.tile([128, EC, cr * 512], F16, tag="xt", name="xt",
                                padded_shape=[128, EC, SC])
                # halves on different DMA rings so the first L1 matmul's
                # input lands in half the time
                for e in range(EC):
                    nc.sync.dma_start(
                        xt_t[:, e, 0:512],
                        xt_d.ap()[e * 128:(e + 1) * 128, base:base + 512])
                if cr > 1:
                    nc.gpsimd.dma_start(
                        xt_t[:, :, 512:1024],
                        xt_d.ap()[:, base + 512:base + 1024]
                            .rearrange("(e p) t -> p e t", p=128))
                sel_t = selp.tile([128, cr * 4, 128], F16, tag="sel",
                                  name="sel", padded_shape=[128, QC, 128])
                nc.sync.dma_start(
                    sel_t[:],
                    sel_d.ap()[base:base + cr * 512, :]
                         .rearrange("(q p) s -> p q s", p=128))
                h1_t = actp.tile([128, HC, cr, 512], F16, tag="h1", name="h1",
                                 bufs=3, padded_shape=[128, HC, 2, 512])
                for h in range(HC):
                    for c in range(cr):
                        ps1 = psp1.tile([128, 512], F32, tag="l1", name="l1",
                                        bufs=5)
                        for e in range(EC):
                            _mm(nc, ps1[:], w1_sb[e][h],
                                xt_t[:, e, c * 512:(c + 1) * 512],
                                start=(e == 0), stop=(e == EC - 1), skip=True)
                        nc.scalar.activation(h1_t[:, h, c, :], ps1[:],
                                             mybir.ActivationFunctionType.Tanh,
                                             bias=b1_sb[:, h:h + 1])
                return sel_t, h1_t, cr

            def l2(sel_t, h1_t, cr):
                z2_t = actp.tile([128, cr, 4, 512], F16, tag="z2", name="z2",
                                 bufs=2, padded_shape=[128, 2, 4, 512])
                for c in range(cr):
                    for ts in range(4):
                        ps2 = psp2.tile([128, 512], F32, tag="z2p",
                                        name="z2p", bufs=2)
                        for k in range(HC):
                            _mm(nc, ps2[:],
                                h1_t[:, k, c, ts * 128:(ts + 1) * 128],
                                w2_mv[k],
                                start=(k == 0), stop=(k == HC - 1), skip=True)
                        nc.vector.scalar_tensor_tensor(
                            z2_t[:, c, ts, :], ps2[:], 1.0, b2_sb[:],
                            op0=mybir.AluOpType.mult,
                            op1=mybir.AluOpType.add)
                h2_t = actp.tile([128, cr, 4, 512], F16, tag="h2", name="h2",
                                 bufs=3, padded_shape=[128, 2, 4, 512])
                nc.scalar.activation(h2_t[:], z2_t[:],
                                     mybir.ActivationFunctionType.Tanh)
                return sel_t, h2_t, cr

            def seg(sel_t, h2_t, cr, is_last):
                for c in range(cr):
                    for ts in range(4):
                        q = c * 4 + ts
                        last = is_last and q == cr * 4 - 1
                        _mm(nc, enc_ps[:], sel_t[:, q, :], h2_t[:, c, ts, :],
                            start=False, stop=last, skip=True)

            pend1 = []
            pend2 = []
            for ui, (base, cr) in enumerate(units):
                pend1.append(load_and_l1(base, cr))
                if ui == 0:
                    nc.gpsimd.dma_start(b2_sb[:], b2_d.ap())
                if len(pend1) > 1:
                    pend2.append(l2(*pend1.pop(0)))
                if len(pend2) > 1:
                    seg(*pend2.pop(0), is_last=False)
            while pend1:
                pend2.append(l2(*pend1.pop(0)))
                if len(pend2) > 1:
                    seg(*pend2.pop(0), is_last=False)
            seg(*pend2.pop(0), is_last=True)

            nc.gpsimd.dma_start(pb1_sb[:], pb1_d.ap().rearrange("h p -> p h"))
            nc.gpsimd.dma_start(pb2_sb[:], pb2_d.ap().rearrange("h p -> p h"))
            nc.gpsimd.dma_start(b3row[:], b3_d.ap())
            nc.gpsimd.dma_start(pb3row[:], pb3_d.ap())
            nc.gpsimd.dma_start(cntrow[:], cnt_d.ap())
            p1_t = wp.tile([128, CC, HC, 128], F32R, tag="p1", name="p1t")
            nc.gpsimd.dma_start(
                p1_t[:], p1_d.ap().rearrange("(c p) (h q) -> p c h q",
                                             p=128, q=128))
            p1_sb = [[p1_t[:, c, h, :] for h in range(HC)] for c in range(CC)]
            p2_t = wp.tile([128, HC, HC, 128], F32R, tag="p2", name="p2t")
            nc.gpsimd.dma_start(
                p2_t[:], p2_d.ap().rearrange("(k p) (h q) -> p k h q",
                                             p=128, q=128))
            p2_sb = [[p2_t[:, k, h, :] for h in range(HC)] for k in range(HC)]
            p3_t = wp.tile([128, HC, O], F32R, tag="p3", name="p3t")
            nc.gpsimd.dma_start(
                p3_t[:], p3_d.ap().rearrange("(k p) o -> p k o", p=128))
            p3_sb = [p3_t[:, k, :] for k in range(HC)]

            # ---- epilogue: enc -> te[c, slot] -> pred[slot, O] ----
            enc_sb = smp.tile([NS, H], F32, tag="encsb", name="encsb")
            nc.vector.tensor_copy(enc_sb[:], enc_ps[:])
            # transpose to feature-major [128, HC, NS] (f16 for W3 matmul)
            encTf = smp.tile([128, HC, SLOTS], F16, tag="encTf",
                             name="encTf")
            for k in range(HC):
                pst = psp2.tile([128, NS], F32, tag="z2p", name="pst",
                                bufs=2)
                nc.tensor.transpose(pst[:], enc_sb[:, k * 128:(k + 1) * 128],
                                    ident[:])
                nc.vector.tensor_copy(encTf[:, k, :], pst[:, 0:SLOTS])
            # te[c, slot] = sum_k W3[k, c] * encT[k, slot] + b3[c]*cnt[slot]
            psW3 = psp1.tile([128, CC, SLOTS], F32, tag="l1", name="psW3",
                             bufs=5)
            for ct in range(CC):
                nc.tensor.matmul(psW3[:, ct, 0:SLOTS],
                                 b3row[:, ct * 128:(ct + 1) * 128],
                                 cntrow[:],
                                 start=(ct == 0), stop=False,
                                 skip_group_check=True)
            for ct in range(CC):
                for k in range(HC):
                    _mm(nc, psW3[:, ct, :],
                        w3_t[:, k, ct * 128:(ct + 1) * 128],
                        encTf[:, k, :],
                        start=False, stop=(k == HC - 1), skip=True)
            # te (psW3) straight to f32r for the predictor matmuls
            encT_sb = smp.tile([128, CC, SLOTS], F32R, tag="encT", name="encT")
            for ct in range(CC):
                nc.vector.tensor_copy(encT_sb[:, ct, :], psW3[:, ct, :])

            # ---- predictor on this core's own <=SLOTS segment rows ----
            q1_sb = smp.tile([128, HC, SLOTS], F32R, tag="q1", name="q1")
            for h in range(HC):
                pp1 = psp2.tile([128, SLOTS], F32, tag="z2p", name="pp1",
                                bufs=2)
                for c in range(CC):
                    _mm(nc, pp1[:], p1_sb[c][h], encT_sb[:, c, :],
                        start=(c == 0), stop=(c == CC - 1))
                nc.scalar.activation(q1_sb[:, h, :], pp1[:],
                                     mybir.ActivationFunctionType.Tanh,
                                     bias=pb1_sb[:, h:h + 1])
            q2_sb = smp.tile([128, HC, SLOTS], F32R, tag="q2", name="q2")
            for h in range(HC):
                pp2 = psp2.tile([128, SLOTS], F32, tag="z2p", name="pp2",
                                bufs=2)
                for k in range(HC):
                    _mm(nc, pp2[:], p2_sb[k][h], q1_sb[:, k, :],
                        start=(k == 0), stop=(k == HC - 1))
                nc.scalar.activation(q2_sb[:, h, :], pp2[:],
                                     mybir.ActivationFunctionType.Tanh,
                                     bias=pb2_sb[:, h:h + 1])

            # final: pred[slot, o] = q2.T @ P3 + pb3
            ppo = psp2.tile([SLOTS, O], F32, tag="z2p", name="ppo", bufs=2)
            nc.tensor.matmul(ppo[:], ones1[:], pb3row[:],
                             start=True, stop=False, skip_group_check=True)
            for k in range(HC):
                _mm(nc, ppo[:], q2_sb[:, k, :], p3_sb[k],
                    start=False, stop=(k == HC - 1), skip=True)
            pred_sb = smp.tile([SLOTS, O], F32, tag="pred", name="predsb")
            nc.vector.tensor_copy(pred_sb[:], ppo[:])
            nc.sync.dma_start(out_d.ap(), pred_sb[:])

    nc.compile()
    return nc


def kernel(words, seg_ids, W1, b1, W2, b2, W3, b3,
           P1, pb1, P2, pb2, P3, pb3, batch_size, alpha_iter, **_):
    words = np.asarray(words, dtype=np.float32)
    seg_ids = np.asarray(seg_ids).astype(np.int64)
    assert words.shape == (T, E), words.shape
    bs, ai = int(batch_size), int(alpha_iter)

    # --- host-side index prep: cut the sorted token axis at segment
    # boundaries so each core owns whole segments ---
    counts = np.bincount(seg_ids, minlength=S)[:S]
    starts = np.concatenate([[0], np.cumsum(counts)])   # [S+1]
    cuts = [0]
    for c in range(1, N_CORES):
        tgt = c * T // N_CORES
        j = int(np.searchsorted(starts, tgt, side="left"))
        if j > 0 and tgt - starts[j - 1] < starts[j] - tgt:
            j -= 1
        cuts.append(int(starts[j]))
    cuts.append(T)
    lens = np.diff(cuts)
    t_sh = int(np.ceil(lens.max() / 512) * 512)

    # contiguous segment range owned by each core (covers all of [0, S));
    # empty shards inherit the following shard's start so ranges stay
    # monotone and collectively exhaustive
    seg_lo = [0] * N_CORES
    for c in range(N_CORES - 1, 0, -1):
        if lens[c] > 0:
            seg_lo[c] = int(seg_ids[cuts[c]])
        else:
            seg_lo[c] = S if c == N_CORES - 1 else seg_lo[c + 1]
    seg_hi = seg_lo[1:] + [S]
    slots_needed = max(seg_hi[c] - seg_lo[c] for c in range(N_CORES))
    SLOTS = min(128, max(MIN_SLOTS, ((slots_needed + 31) // 32) * 32))
    assert slots_needed <= SLOTS, (seg_lo, seg_hi)
    assert bs * ai == S

    xt = np.ascontiguousarray(words.T.astype(np.float16))    # [E, T] fp16

    key = ("nc", t_sh, SLOTS)
    if key not in _CACHE:
        _CACHE[key] = _build_nc(t_sh, SLOTS)
    nc = _CACHE[key]

    common = {
        "w1": np.ascontiguousarray(W1, dtype=np.float16),
        "w2": np.ascontiguousarray(W2, dtype=np.float16),
        "w3": np.ascontiguousarray(W3, dtype=np.float16),
        "b1": np.ascontiguousarray(b1, dtype=np.float32).reshape(H // 128, 128),
        "b2": np.ascontiguousarray(
            np.broadcast_to(np.asarray(b2, dtype=np.float32)[None, :],
                            (128, H))),
        "b3": np.ascontiguousarray(b3, dtype=np.float32).reshape(1, C),
        "p1": np.ascontiguousarray(P1, dtype=np.float32),
        "p2": np.ascontiguousarray(P2, dtype=np.float32),
        "p3": np.ascontiguousarray(P3, dtype=np.float32),
        "pb1": np.ascontiguousarray(pb1, dtype=np.float32).reshape(H // 128, 128),
        "pb2": np.ascontiguousarray(pb2, dtype=np.float32).reshape(H // 128, 128),
        "pb3": np.ascontiguousarray(pb3, dtype=np.float32).reshape(1, O),
    }
    in_maps = []
    for c in range(N_CORES):
        lo, hi = cuts[c], cuts[c + 1]
        n = hi - lo
        xt_c = np.zeros((E, t_sh), dtype=np.float16)
        xt_c[:, :n] = xt[:, lo:hi]
        sel_c = np.zeros((t_sh, 128), dtype=np.float16)
        sel_c[:n, :SLOTS] = (seg_ids[lo:hi, None] ==
                             (seg_lo[c] + np.arange(SLOTS))[None, :])
        cnt_c = np.zeros((1, SLOTS), dtype=np.float32)
        nseg = seg_hi[c] - seg_lo[c]
        cnt_c[0, :nseg] = counts[seg_lo[c]:seg_hi[c]]
        in_maps.append({
            **common,
            "xt": xt_c,
            "sel": sel_c,
            "cnt": cnt_c,
        })

    global _LAST_IN_MAPS
    _LAST_IN_MAPS = in_maps
    res = bass_utils.run_bass_kernel_spmd(nc, in_maps,
                                          core_ids=list(range(N_CORES)))
    pred = np.zeros((S, O), dtype=np.float32)
    for c in range(N_CORES):
        nseg = seg_hi[c] - seg_lo[c]
        if nseg > 0:
            pred[seg_lo[c]:seg_hi[c]] = res.results[c]["pred"][:nseg]
    return pred.reshape(bs, ai, O).astype(np.float32)


_LAST_IN_MAPS = None
